# revision 3
# baseline (speedup 1.0000x reference)
"""AdaConv Trainium2 kernel: instance-norm + per-sample depthwise 3x3 (+scale+bias)
+ shared dense 3x3 conv 256->1024, data-parallel over batch on 8 NeuronCores.

v2 layout (per core = one sample), activations/weights bf16 on the PE path:
  - xt [2][128 ch, 66, 66] bf16 with zero 1-pixel border (host-padded).
    xt[0] is DMA'd in 3 pieces (rows 0:11 / 11:33 / 33:66) so the raw
    depthwise chunk 0 can start as soon as the first 93 KB lands.
  - Raw depthwise 3x3 on the DVE with UNNORMALIZED weights w (no stats
    dependency): rv = x (*) w accumulated in av bf16. Normalization is
    deferred: av = iv*rv + bt on the ACT engine per chunk, where
    iv = 1/(std+eps), bt = bias - mean*iv*sum(w). Borders stay zero.
    (The per-edge-pixel mean-leak correction term is ~1e-4 relative and
    is deliberately dropped; measured end-to-end rel err ~5e-3.)
  - Stats: sum on DVE tensor_reduce per DMA piece; sum-of-squares on ACT
    Square+accum_out per piece; combined into mean/var -> iv/bt.
  - Depthwise chunks: [1,10) gates the first psum group; then 16-row
    chunks [10,26),[26,42),[42,58),[58,65) to amortize DVE op overhead.
  - Shared 3x3 conv: per pixel-row-block pc, 8 psum banks accumulate 2x9
    tap matmuls (bf16, 512-px moving dim = full PE rate); cb0 taps for
    all 8 ob blocks are emitted before the cb1 taps.
  - ACT evacuates psum (+conv_b) to f32; DMA out o-major [1024, 4096];
    the final row-block's output DMAs are split 4-way to shorten the tail.
Host re-lays-out the o-major per-core outputs into [8, 64, 64, 1024].
"""

import os

import numpy as np

import concourse.bacc as bacc
import concourse.mybir as mybir
import concourse.tile as tile
from concourse.bass_utils import run_bass_kernel_spmd

F32 = mybir.dt.float32
BF16 = mybir.dt.bfloat16
AF = mybir.ActivationFunctionType
ALU = mybir.AluOpType

B = 8
H = W = 64
C = 256
CB = 2
OUT = 1024
PW = 66
NPIX = H * W
EPS = 1e-5
N_OB = OUT // 128
N_PC = H // 8

# depthwise chunk row ranges (padded dst rows)
DW_CHUNKS = [(1, 10), (10, 26), (26, 42), (42, 58), (58, 65)]
# xt[0] DMA row pieces
XT0_PIECES = [(0, 11), (11, 33), (33, 66)]
XT1_PIECES = [(0, 33), (33, 66)]


def build_nc(compile: bool = True):
    nc = bacc.Bacc("TRN2", target_bir_lowering=False, debug=False)

    xt_d = nc.dram_tensor("xt", [CB, 128, PW, PW], BF16, kind="ExternalInput").ap()
    wv_d = nc.dram_tensor("wv", [CB, 128, 9], F32, kind="ExternalInput").ap()
    bias_d = nc.dram_tensor("bias", [CB, 128, 1], F32, kind="ExternalInput").ap()
    cwt_d = nc.dram_tensor("cwt", [CB, 128, 9, OUT], BF16, kind="ExternalInput").ap()
    cbt_d = nc.dram_tensor("cbt", [128, N_OB], F32, kind="ExternalInput").ap()
    out_d = nc.dram_tensor("out", [OUT, NPIX], F32, kind="ExternalOutput").ap()

    with tile.TileContext(nc) as tc:
        with (
            tc.tile_pool(name="res", bufs=1) as RP,
            tc.tile_pool(name="psc", bufs=8, space="PSUM") as PSC,
            tc.tile_pool(name="outp", bufs=4) as OP,
            tc.tile_pool(name="small", bufs=1) as SP,
        ):
            xt = [RP.tile([128, PW, PW], BF16, name=f"xt{i}", tag=f"xt{i}") for i in range(CB)]
            av = [RP.tile([128, PW, PW], BF16, name=f"av{i}", tag=f"av{i}") for i in range(CB)]
            cw = [RP.tile([128, 9, OUT], BF16, name=f"cw{i}", tag=f"cw{i}") for i in range(CB)]
            scr = RP.tile([128, 33, PW], F32, name="scr", tag="scr")
            wv = [SP.tile([128, 9], F32, name=f"wv{i}", tag=f"wv{i}") for i in range(CB)]
            bi = [SP.tile([128, 1], F32, name=f"bi{i}", tag=f"bi{i}") for i in range(CB)]
            cbt = SP.tile([128, N_OB], F32, name="cbt", tag="cbt")
            warm = SP.tile([128, 1], F32, name="warm", tag="warm")
            warm2 = SP.tile([128, 1], F32, name="warm2", tag="warm2")

            # ACT table warm-up (Sqrt + Identity) while DMAs stream in.
            nc.gpsimd.memset(warm[:, :], 0.0)
            nc.scalar.activation(out=warm2[:, :], in_=warm[:, :], func=AF.Sqrt)
            nc.scalar.activation(out=warm2[:, :], in_=warm[:, :], func=AF.Identity)

            # av borders must read as zero in the shared conv.
            for cb in range(CB):
                nc.gpsimd.memset(av[cb][:, 0 : PW : 65, :], 0.0)
                nc.gpsimd.memset(av[cb][:, :, 0 : PW : 65], 0.0)

            # DMA priority: small scalars, xt0 pieces, cw0 ob0, xt1, rest of cw.
            for cb in range(CB):
                nc.sync.dma_start(out=wv[cb][:, :], in_=wv_d[cb])
                nc.sync.dma_start(out=bi[cb][:, :], in_=bias_d[cb])
            nc.sync.dma_start(out=cbt[:, :], in_=cbt_d)
            for r0, r1 in XT0_PIECES:
                nc.sync.dma_start(out=xt[0][:, r0:r1, :], in_=xt_d[0][:, r0:r1, :])
            nc.sync.dma_start(
                out=cw[0][:, :, 0:128], in_=cwt_d[0][:, :, 0:128]
            )
            for r0, r1 in XT1_PIECES:
                nc.sync.dma_start(out=xt[1][:, r0:r1, :], in_=xt_d[1][:, r0:r1, :])
            for ob in range(1, N_OB):
                nc.sync.dma_start(
                    out=cw[0][:, :, ob * 128 : (ob + 1) * 128],
                    in_=cwt_d[0][:, :, ob * 128 : (ob + 1) * 128],
                )
            for ob in range(N_OB):
                nc.sync.dma_start(
                    out=cw[1][:, :, ob * 128 : (ob + 1) * 128],
                    in_=cwt_d[1][:, :, ob * 128 : (ob + 1) * 128],
                )

            # per-channel sum of the 9 depthwise taps (for bt)
            wa = [SP.tile([128, 1], F32, name=f"wa{cb}", tag=f"wa{cb}") for cb in range(CB)]
            for cb in range(CB):
                nc.vector.tensor_reduce(
                    out=wa[cb][:, :], in_=wv[cb][:, :], axis=mybir.AxisListType.X,
                    op=ALU.add,
                )

            iv = [SP.tile([128, 1], F32, name=f"iv{c}", tag=f"iv{c}") for c in range(CB)]
            bt = [SP.tile([128, 1], F32, name=f"bt{c}", tag=f"bt{c}") for c in range(CB)]

            def dw_chunk_raw(k, cb):
                """Raw (unnormalized) 9-tap depthwise accumulation for chunk k."""
                r0, r1 = DW_CHUNKS[k]
                dst = av[cb][:, r0:r1, 1:65]
                for t in range(9):
                    ty, tx = t // 3, t % 3
                    src = xt[cb][:, r0 + ty - 1 : r1 + ty - 1, tx : tx + 64]
                    if t == 0:
                        nc.vector.tensor_scalar_mul(dst, src, wv[cb][:, 0:1])
                    else:
                        nc.vector.scalar_tensor_tensor(
                            out=dst, in0=src, scalar=wv[cb][:, t : t + 1],
                            in1=dst, op0=ALU.mult, op1=ALU.add,
                        )

            def dw_scale(k, cb):
                """Deferred normalization: av = iv*raw + bt (ACT engine)."""
                r0, r1 = DW_CHUNKS[k]
                dst = av[cb][:, r0:r1, 1:65]
                nc.scalar.activation(
                    out=dst, in_=dst, func=AF.Identity,
                    bias=bt[cb][:, :], scale=iv[cb][:, :],
                )

            # ---- stats scratch ----
            npieces = [len(XT0_PIECES), len(XT1_PIECES)]
            ssum = [SP.tile([128, npieces[c]], F32, name=f"ssum{c}", tag=f"ssum{c}") for c in range(CB)]
            ssq = [SP.tile([128, npieces[c]], F32, name=f"ssq{c}", tag=f"ssq{c}") for c in range(CB)]
            mean = [SP.tile([128, 1], F32, name=f"mean{c}", tag=f"mean{c}") for c in range(CB)]
            e2 = [SP.tile([128, 1], F32, name=f"e2{c}", tag=f"e2{c}") for c in range(CB)]
            msq = [SP.tile([128, 1], F32, name=f"msq{c}", tag=f"msq{c}") for c in range(CB)]
            var = [SP.tile([128, 1], F32, name=f"var{c}", tag=f"var{c}") for c in range(CB)]
            std = [SP.tile([128, 1], F32, name=f"std{c}", tag=f"std{c}") for c in range(CB)]
            sd = [SP.tile([128, 1], F32, name=f"sd{c}", tag=f"sd{c}") for c in range(CB)]
            nm = [SP.tile([128, 1], F32, name=f"nm{c}", tag=f"nm{c}") for c in range(CB)]
            s1 = [SP.tile([128, 1], F32, name=f"s1{c}", tag=f"s1{c}") for c in range(CB)]
            q1 = [SP.tile([128, 1], F32, name=f"q1{c}", tag=f"q1{c}") for c in range(CB)]

            def fold(cb):
                """mean/var -> iv, bt (DVE small ops + one ACT sqrt)."""
                nc.vector.tensor_reduce(
                    out=s1[cb][:, :], in_=ssum[cb][:, :], axis=mybir.AxisListType.X,
                    op=ALU.add,
                )
                nc.vector.tensor_reduce(
                    out=q1[cb][:, :], in_=ssq[cb][:, :], axis=mybir.AxisListType.X,
                    op=ALU.add,
                )
                nc.vector.tensor_scalar_mul(mean[cb][:, :], s1[cb][:, :], 1.0 / NPIX)
                nc.vector.tensor_scalar_mul(e2[cb][:, :], q1[cb][:, :], 1.0 / NPIX)
                nc.vector.tensor_mul(msq[cb][:, :], mean[cb][:, :], mean[cb][:, :])
                nc.vector.tensor_sub(var[cb][:, :], e2[cb][:, :], msq[cb][:, :])
                nc.scalar.activation(out=std[cb][:, :], in_=var[cb][:, :], func=AF.Sqrt)
                nc.vector.tensor_scalar_add(sd[cb][:, :], std[cb][:, :], EPS)
                nc.vector.reciprocal(iv[cb][:, :], sd[cb][:, :])
                nc.vector.tensor_scalar(
                    out=nm[cb][:, :], in0=mean[cb][:, :], scalar1=iv[cb][:, :],
                    scalar2=-1.0, op0=ALU.mult, op1=ALU.mult,
                )
                nc.vector.scalar_tensor_tensor(
                    out=bt[cb][:, :], in0=nm[cb][:, :], scalar=wa[cb][:, :],
                    in1=bi[cb][:, :], op0=ALU.mult, op1=ALU.add,
                )

            # ---- cb0 pipeline: taps chunk0 -> stats -> fold -> scale chunk0 ----
            # ACT: sumsq per xt0 piece (parallel with DVE taps)
            for j, (r0, r1) in enumerate(XT0_PIECES):
                nc.scalar.activation(
                    out=scr[:, 0 : r1 - r0, :], in_=xt[0][:, r0:r1, :],
                    func=AF.Square, accum_out=ssq[0][:, j : j + 1],
                )
            # DVE: chunk0 raw taps first (they gate the first matmul), then sums
            dw_chunk_raw(0, 0)
            for j, (r0, r1) in enumerate(XT0_PIECES):
                nc.vector.tensor_reduce(
                    out=ssum[0][:, j : j + 1], in_=xt[0][:, r0:r1, :],
                    axis=mybir.AxisListType.XY, op=ALU.add,
                )
            fold(0)
            with tc.high_priority():
                dw_scale(0, 0)

            # ---- cb1 stats (ACT squares emitted after the cb0 scale) ----
            for j, (r0, r1) in enumerate(XT1_PIECES):
                nc.scalar.activation(
                    out=scr[:, 0 : r1 - r0, :], in_=xt[1][:, r0:r1, :],
                    func=AF.Square, accum_out=ssq[1][:, j : j + 1],
                )
            dw_chunk_raw(0, 1)
            for j, (r0, r1) in enumerate(XT1_PIECES):
                nc.vector.tensor_reduce(
                    out=ssum[1][:, j : j + 1], in_=xt[1][:, r0:r1, :],
                    axis=mybir.AxisListType.XY, op=ALU.add,
                )
            fold(1)
            dw_scale(0, 1)

            for k in range(1, len(DW_CHUNKS)):
                for cb in range(CB):
                    dw_chunk_raw(k, cb)
                    dw_scale(k, cb)

            # ---- shared 3x3 conv 256 -> 1024 (+conv_b) on the PE ----
            for pc in range(N_PC):
                ps = [
                    PSC.tile([128, 8, 64], F32, name=f"psc{ob}_{pc}", tag="psc")
                    for ob in range(N_OB)
                ]
                for cb in range(CB):
                    for ob in range(N_OB):
                        for t in range(9):
                            ty, tx = t // 3, t % 3
                            nc.tensor.matmul(
                                out=ps[ob][:, :, :],
                                lhsT=cw[cb][:, t, ob * 128 : (ob + 1) * 128],
                                rhs=av[cb][:, 8 * pc + ty : 8 * pc + ty + 8, tx : tx + 64],
                                start=(cb == 0 and t == 0),
                                stop=(cb == 1 and t == 8),
                            )
                        if cb == 1:
                            ot = OP.tile([128, 8, 64], F32, name=f"ot{ob}_{pc}", tag="ot")
                            nc.scalar.activation(
                                out=ot[:, :, :], in_=ps[ob][:, :, :],
                                func=AF.Identity, bias=cbt[:, ob : ob + 1], scale=1.0,
                            )
                            orow = out_d[ob * 128 : (ob + 1) * 128]
                            if pc == N_PC - 1:
                                for q in range(4):
                                    nc.sync.dma_start(
                                        out=orow[:, 512 * pc + 128 * q : 512 * pc + 128 * (q + 1)],
                                        in_=ot[:, 2 * q : 2 * (q + 1), :],
                                    )
                            else:
                                nc.sync.dma_start(
                                    out=orow[:, 512 * pc : 512 * (pc + 1)],
                                    in_=ot[:, :, :],
                                )

    if compile:
        nc.compile()
    return nc


def make_in_maps(inputs: dict) -> list[dict]:
    import ml_dtypes

    x = np.asarray(inputs["x"], dtype=np.float32)
    ws = np.asarray(inputs["w_spatial"], dtype=np.float32)
    wp = np.asarray(inputs["w_pointwise"], dtype=np.float32)
    bias = np.asarray(inputs["bias"], dtype=np.float32)
    conv_w = np.asarray(inputs["conv_w"], dtype=np.float32)
    conv_b = np.asarray(inputs["conv_b"], dtype=np.float32)

    xt = np.zeros((B, CB, 128, PW, PW), np.float32)
    xt[:, :, :, 1:65, 1:65] = x.transpose(0, 3, 1, 2).reshape(B, CB, 128, H, W)
    xt = xt.astype(ml_dtypes.bfloat16)

    weff = (ws[:, :, :, 0, :] * wp[:, 0, 0, 0, :][:, None, None, :]).reshape(B, 9, C)
    wv = np.ascontiguousarray(weff.reshape(B, 9, CB, 128).transpose(0, 2, 3, 1))

    bias_r = np.ascontiguousarray(bias.reshape(B, CB, 128, 1))
    cwt = conv_w.reshape(9, CB, 128, OUT).transpose(1, 2, 0, 3)
    cwt = np.ascontiguousarray(cwt.astype(ml_dtypes.bfloat16))
    cbt = np.ascontiguousarray(conv_b.reshape(N_OB, 128).T)

    return [
        {
            "xt": np.ascontiguousarray(xt[b]),
            "wv": wv[b],
            "bias": bias_r[b],
            "cwt": cwt,
            "cbt": cbt,
        }
        for b in range(B)
    ]


def gather(results: list[dict]) -> np.ndarray:
    outs = []
    for b in range(B):
        o = np.asarray(results[b]["out"])
        outs.append(o.reshape(OUT, H, W).transpose(1, 2, 0))
    return np.ascontiguousarray(np.stack(outs).astype(np.float32))


_STATE = {}


def _get_nc():
    if "nc" not in _STATE:
        _STATE["nc"] = build_nc()
    return _STATE["nc"]


def kernel(**inputs) -> np.ndarray:
    nc = _get_nc()
    in_maps = make_in_maps(inputs)
    last_err = None
    for _attempt in range(3):
        try:
            res = run_bass_kernel_spmd(nc, in_maps, core_ids=list(range(B)))
            return gather(res.results)
        except Exception as e:  # transient device-unrecoverable seen on 1st exec
            last_err = e
    raise last_err
